# revision 1
# baseline (speedup 1.0000x reference)
"""Trainium2 Bass kernel for nn_ContextualEncoder2 (5-step GRU over buoys).

Strategy (data-parallel over 16384 buoys across 8 cores, 2048 each):
  * Transposed compute layout: gate-features on SBUF partitions, buoys on
    the free axis. h stays [H, cols] in SBUF between steps -> no transposes.
  * gates.T = W.T-tiles (stationary, fp32r) @ h.T-tiles (moving, fp32r),
    accumulated in PSUM [128, 512] tiles. All contractions are K=128
    (obs/onehot operands are zero-padded on host so no slow partial-row
    matmuls appear in the PE stream).
  * The embedding gather emb[ids] @ W_ih[:,64:].T is algebraically replaced
    by a onehot(ids) matmul against emb_proj = emb @ W_ih[:,64:].T (one
    extra K=128 matmul per PSUM tile). emb_proj is computed on device once.
  * Step 5 uses W_hh + W_ih[:, :1024] summed on host (its gi and gh parts
    both consume h4), saving one full contraction for the r/z gates.
  * All biases are applied as per-partition ACT bias operands.
  * outs[0] (h after step 1) is spilled to DRAM and streamed back in step 4.
  * Block schedule is software-pipelined: block b+1's latency-bound step 1
    is emitted before block b's step 5 so its chains hide under PE work.
"""
import numpy as np

import concourse.bass as bass
import concourse.mybir as mybir
import concourse.tile as tile
from concourse import bacc
from concourse.bass_utils import run_bass_kernel_spmd

F32 = mybir.dt.float32
F32R = mybir.dt.float32r
AF = mybir.ActivationFunctionType
OP = mybir.AluOpType

N_CORES = 8
NUM_BUOYS = 16384
H = 1024
G3 = 3072
NEMB = 100
KCH = 8          # 1024 / 128 contraction chunks
FCH = 8          # 1024 / 128 gate-feature tiles
NT = 512         # moving/free tile width (one PSUM bank of fp32)
NS = G3 // NT    # emb_proj column slices


def _accum(nc, psum, pairs):
    last = len(pairs) - 1
    for i, (l, r) in enumerate(pairs):
        nc.tensor.matmul(psum, l, r, start=(i == 0), stop=(i == last))


def build(nbuoy=2048, blk=1024):
    """Build the per-core Bass program (same NEFF on every core)."""
    assert nbuoy % blk == 0 and blk % NT == 0
    nblk = nbuoy // blk
    J = blk // NT

    nc = bacc.Bacc("TRN2", target_bir_lowering=False, debug=False)

    whh = nc.declare_dram_parameter("whh", [24, 128, 1024], F32R, isOutput=False)
    wih = nc.declare_dram_parameter("wih", [24, 128, 1024], F32R, isOutput=False)
    wsum = nc.declare_dram_parameter("wsum", [16, 128, 1024], F32R, isOutput=False)
    wemb = nc.declare_dram_parameter("wemb", [KCH, 128, G3], F32R, isOutput=False)
    wobs = nc.declare_dram_parameter("wobs", [128, G3], F32R, isOutput=False)
    embt = nc.declare_dram_parameter("embt", [KCH, 128, NEMB], F32R, isOutput=False)
    # onehot rows 100..127 are zero; obs tiles carry a zero half (see prep)
    onehot = nc.declare_dram_parameter("onehot", [128, nbuoy], F32R, isOutput=False)
    obs0 = nc.declare_dram_parameter("obs0", [128, nbuoy], F32R, isOutput=False)
    obs23 = nc.declare_dram_parameter("obs23", [2, 128, nbuoy], F32R, isOutput=False)
    obs45 = nc.declare_dram_parameter("obs45", [2, 128, nbuoy], F32R, isOutput=False)
    bih = nc.declare_dram_parameter("bih", [128, 24], F32, isOutput=False)
    bhh = nc.declare_dram_parameter("bhh", [128, 24], F32, isOutput=False)
    out_t = nc.declare_dram_parameter("out_t", [FCH, 128, nbuoy], F32, isOutput=True)

    whh_ap, wih_ap, wsum_ap, wemb_ap = whh.ap(), wih.ap(), wsum.ap(), wemb.ap()
    out_ap = out_t.ap()
    obs23_ap, obs45_ap = obs23.ap(), obs45.ap()

    with tile.TileContext(nc) as tc:
        with (
            tc.tile_pool(name="const", bufs=1) as cpool,
            tc.tile_pool(name="obsl", bufs=1) as opool,
            tc.tile_pool(name="htiles", bufs=1) as hpool,
            tc.tile_pool(name="work", bufs=2) as wpool,
        ):
            emb_proj = cpool.tile([128, G3], F32R, tag="embproj")
            nc.gpsimd.memset(emb_proj[96:128, :].bitcast(F32), 0.0)

            # ---- phase A: emb_proj[:100] = emb @ W_ih[:, 64:1088].T --------
            # (emitted before the constant loads so its 12.6MB weight stream
            #  heads the DMA queues -- everything else waits on it anyway)
            with (
                tc.tile_pool(name="phA", bufs=2) as apool,
                tc.tile_pool(name="psA", bufs=1, space="PSUM") as psA,
            ):
                with nc.named_scope("embproj"):
                    emb_sb = apool.tile([128, KCH * NEMB], F32R, tag="embt")
                    for k in range(KCH):
                        nc.sync.dma_start(
                            emb_sb[:, k * NEMB:(k + 1) * NEMB], embt.ap()[k])
                    psums = [psA.tile([NEMB, NT], F32, tag=f"embp{s}",
                                      name=f"embp{s}") for s in range(NS)]
                    for k in range(KCH):
                        wk = apool.tile([128, G3], F32R, tag="wemb", name="wk")
                        nc.sync.dma_start(wk[:], wemb_ap[k])
                        for s in range(NS):
                            nc.tensor.matmul(
                                psums[s][:],
                                emb_sb[:, k * NEMB:(k + 1) * NEMB],
                                wk[:, s * NT:(s + 1) * NT],
                                start=(k == 0), stop=(k == KCH - 1))
                    for s in range(NS):
                        nc.scalar.activation(
                            emb_proj[:NEMB, s * NT:(s + 1) * NT], psums[s][:],
                            AF.Copy)

            # ---- constants -------------------------------------------------
            bih_sb = cpool.tile([128, 24], F32, tag="bih")
            nc.sync.dma_start(bih_sb[:], bih.ap())
            bhh_sb = cpool.tile([128, 24], F32, tag="bhh")
            nc.sync.dma_start(bhh_sb[:], bhh.ap())
            bsum = cpool.tile([128, 24], F32, tag="bsum")
            nc.vector.tensor_add(bsum[:], bih_sb[:], bhh_sb[:])
            wobs_sb = cpool.tile([128, G3], F32R, tag="wobs")
            nc.sync.dma_start(wobs_sb[:], wobs.ap())

            # ---- phase B ---------------------------------------------------
            with (
                tc.tile_pool(name="wstr", bufs=8) as spool,
                tc.tile_pool(name="psB", bufs=2, space="PSUM") as psB,
            ):
                _rr = [0]
                _tags = ["pr", "pz", "pgh", "pg"]

                def rrtile():
                    t = psB.tile([128, NT], F32, tag=_tags[_rr[0] % 4],
                                 name=f"rr{_rr[0]}")
                    _rr[0] += 1
                    return t

                st = {b: {"h": {}, "h1": {}, "obs": {}} for b in range(nblk)}

                def init_block(b):
                    cb = b * blk
                    o = st[b]["obs"]
                    t = opool.tile([128, blk], F32R, tag="obs0", name="obs0t")
                    nc.sync.dma_start(t[:], obs0.ap()[:, cb:cb + blk])
                    o[1] = t
                    for i, (ap_, tagp) in enumerate(
                            [(obs23_ap, "o23"), (obs45_ap, "o45")]):
                        for s2 in range(2):
                            t = opool.tile([128, blk], F32R,
                                           tag=f"{tagp}_{s2}", name=f"{tagp}{s2}")
                            nc.sync.dma_start(t[:], ap_[s2][:, cb:cb + blk])
                            o[2 + i * 2 + s2] = t   # keys 2,3 (s2/s3), 4,5
                    t = opool.tile([128, blk], F32R, tag="oh", name="oht")
                    nc.sync.dma_start(t[:], onehot.ap()[:, cb:cb + blk])
                    o["oh"] = t

                def step1(b):
                    cb, h, obs = b * blk, st[b]["h"], st[b]["obs"]
                    with nc.named_scope(f"b{b}s1"):
                        for jj in range(J):
                            for f in range(FCH):
                                c0, c1 = jj * NT, (jj + 1) * NT
                                mr, mz, mn = f, 8 + f, 16 + f
                                ps = {}
                                for m, key in ((mr, "r"), (mz, "z"), (mn, "g")):
                                    p = rrtile()
                                    _accum(nc, p[:], [
                                        (wobs_sb[:, m * 128:(m + 1) * 128],
                                         obs[1][:, c0:c1]),
                                        (emb_proj[:, m * 128:(m + 1) * 128],
                                         obs["oh"][:, c0:c1])])
                                    ps[key] = p
                                r = wpool.tile([128, NT], F32, tag="r", name="r")
                                nc.scalar.activation(r[:], ps["r"][:], AF.Sigmoid,
                                                     bias=bsum[:, mr:mr + 1])
                                z = wpool.tile([128, NT], F32, tag="z", name="z")
                                nc.scalar.activation(z[:], ps["z"][:], AF.Sigmoid,
                                                     bias=bsum[:, mz:mz + 1])
                                t2 = wpool.tile([128, NT], F32, tag="t2", name="t2")
                                nc.vector.scalar_tensor_tensor(
                                    t2[:], r[:], bhh_sb[:, mn:mn + 1], ps["g"][:],
                                    OP.mult, OP.add)
                                n_t = wpool.tile([128, NT], F32, tag="n", name="n")
                                nc.scalar.activation(n_t[:], t2[:], AF.Tanh,
                                                     bias=bih_sb[:, mn:mn + 1])
                                v = wpool.tile([128, NT], F32, tag="tmp", name="v")
                                nc.vector.tensor_mul(v[:], z[:], n_t[:])
                                hn = hpool.tile([128, NT], F32R,
                                                tag=f"hA_{f}_{jj}", name="hn")
                                nc.vector.tensor_sub(hn[:], n_t[:], v[:])
                                h[(f, jj)] = hn
                                st[b]["h1"][(f, jj)] = hn

                def stepn(b, s):
                    cb, h, obs = b * blk, st[b]["h"], st[b]["obs"]
                    fam = {2: "hB", 3: "hC", 4: "hB"}.get(s)
                    ot = obs[s]
                    h1t = st[b]["h1"]
                    hnew = {}
                    with nc.named_scope(f"b{b}s{s}"):
                        for f in range(FCH):
                            mr, mz, mn = f, 8 + f, 16 + f

                            def wsl(ap_, m):
                                t = spool.tile([128, 1024], F32R, tag="wsl",
                                               name="wsl")
                                nc.sync.dma_start(t[:], ap_[m])
                                return t

                            wr = wsl(wsum_ap if s == 5 else whh_ap, mr)
                            wz = wsl(wsum_ap if s == 5 else whh_ap, mz)
                            wn = wsl(whh_ap, mn)
                            if s == 4:
                                vr, vz = wsl(wih_ap, mr), wsl(wih_ap, mz)
                            if s >= 4:
                                vn = wsl(wih_ap, mn)
                            for jj in range(J):
                                c0, c1 = jj * NT, (jj + 1) * NT
                                hcol = [h[(k, jj)] for k in range(KCH)]
                                gcol = ([h1t[(k, jj)] for k in range(KCH)]
                                        if s == 4 else hcol)

                                def wmm(w, col):
                                    return [(w[:, k * 128:(k + 1) * 128],
                                             col[k][:]) for k in range(KCH)]

                                pr = psB.tile([128, NT], F32, tag="pr")
                                pairs = wmm(wr, hcol)
                                if s == 4:
                                    pairs += wmm(vr, gcol)
                                pairs.append((wobs_sb[:, mr * 128:(mr + 1) * 128],
                                              ot[:, c0:c1]))
                                if s <= 3:
                                    pairs.append(
                                        (emb_proj[:, mr * 128:(mr + 1) * 128],
                                         obs["oh"][:, c0:c1]))
                                _accum(nc, pr[:], pairs)

                                pz = psB.tile([128, NT], F32, tag="pz")
                                pairs = wmm(wz, hcol)
                                if s == 4:
                                    pairs += wmm(vz, gcol)
                                pairs.append((wobs_sb[:, mz * 128:(mz + 1) * 128],
                                              ot[:, c0:c1]))
                                if s <= 3:
                                    pairs.append(
                                        (emb_proj[:, mz * 128:(mz + 1) * 128],
                                         obs["oh"][:, c0:c1]))
                                _accum(nc, pz[:], pairs)

                                pgh = psB.tile([128, NT], F32, tag="pgh")
                                _accum(nc, pgh[:], wmm(wn, hcol))

                                pg = psB.tile([128, NT], F32, tag="pg")
                                if s <= 3:
                                    pairs = [
                                        (wobs_sb[:, mn * 128:(mn + 1) * 128],
                                         ot[:, c0:c1]),
                                        (emb_proj[:, mn * 128:(mn + 1) * 128],
                                         obs["oh"][:, c0:c1])]
                                else:
                                    pairs = wmm(vn, gcol)
                                    pairs.append(
                                        (wobs_sb[:, mn * 128:(mn + 1) * 128],
                                         ot[:, c0:c1]))
                                _accum(nc, pg[:], pairs)

                                r = wpool.tile([128, NT], F32, tag="r", name="r")
                                nc.scalar.activation(r[:], pr[:], AF.Sigmoid,
                                                     bias=bsum[:, mr:mr + 1])
                                z = wpool.tile([128, NT], F32, tag="z", name="z")
                                nc.scalar.activation(z[:], pz[:], AF.Sigmoid,
                                                     bias=bsum[:, mz:mz + 1])
                                # t1 = (gh_n + b_hh_n) * r
                                t1 = wpool.tile([128, NT], F32, tag="tmp",
                                                name="t1")
                                nc.vector.scalar_tensor_tensor(
                                    t1[:], pgh[:], bhh_sb[:, mn:mn + 1], r[:],
                                    OP.add, OP.mult)
                                t2 = wpool.tile([128, NT], F32, tag="t2",
                                                name="t2")
                                nc.vector.tensor_add(t2[:], t1[:], pg[:])
                                n_t = wpool.tile([128, NT], F32, tag="n", name="n")
                                nc.scalar.activation(n_t[:], t2[:], AF.Tanh,
                                                     bias=bih_sb[:, mn:mn + 1])
                                d = wpool.tile([128, NT], F32, tag="tmp2",
                                               name="d")
                                nc.vector.tensor_sub(
                                    d[:], h[(f, jj)][:].bitcast(F32), n_t[:])
                                e = wpool.tile([128, NT], F32, tag="tmp", name="e")
                                nc.vector.tensor_mul(e[:], z[:], d[:])
                                if s < 5:
                                    hn = hpool.tile([128, NT], F32R,
                                                    tag=f"{fam}_{f}_{jj}",
                                                    name="hn")
                                    nc.vector.tensor_add(hn[:], n_t[:], e[:])
                                    hnew[(f, jj)] = hn
                                else:
                                    ho = wpool.tile([128, NT], F32, tag="hout",
                                                    name="ho")
                                    nc.vector.tensor_add(ho[:], n_t[:], e[:])
                                    nc.sync.dma_start(
                                        out_ap[f][:, cb + c0:cb + c1], ho[:])
                    if s < 5:
                        st[b]["h"] = hnew

                # software-pipelined block schedule
                sched = [(0, 0), (0, 1)]
                for b in range(nblk):
                    sched += [(b, s) for s in (2, 3, 4)]
                    if b + 1 < nblk:
                        sched += [(b + 1, 0), (b + 1, 1)]
                    sched.append((b, 5))

                for b, s in sched:
                    if s == 0:
                        init_block(b)
                    elif s == 1:
                        step1(b)
                    else:
                        stepn(b, s)

    nc.compile()
    return nc


# ---------------------------------------------------------------------------
# host-side prep / sharding
# ---------------------------------------------------------------------------

def _prep_shared(emb, W_ih, W_hh, b_ih, b_hh):
    f = np.float32
    W_ih = np.asarray(W_ih, f)
    W_hh = np.asarray(W_hh, f)

    def slabs(W):  # (3072, 1024) -> [24, 128, 1024]: [m, i, k*128+j] = W[128m+j, 128k+i]
        t = W.reshape(24, 128, 8, 128)          # [m, j, k, i]
        return np.ascontiguousarray(t.transpose(0, 3, 2, 1).reshape(24, 128, 1024))

    whh = slabs(W_hh)
    wih = slabs(W_ih[:, :1024])
    wsum = np.ascontiguousarray(slabs(W_hh + W_ih[:, :1024])[:16])
    # [k, i, n] = W_ih[n, 64 + 128k + i]
    wemb = np.ascontiguousarray(
        W_ih[:, 64:1088].reshape(G3, 8, 128).transpose(1, 2, 0))
    wobs = np.concatenate(
        [W_ih[:, :64].T, W_ih[:, 1024:1088].T], axis=0)  # [128, 3072]
    wobs = np.ascontiguousarray(wobs, f)
    embt = np.ascontiguousarray(np.asarray(emb, f).T.reshape(8, 128, NEMB))
    bih_t = np.ascontiguousarray(np.asarray(b_ih, f).reshape(24, 128).T)
    bhh_t = np.ascontiguousarray(np.asarray(b_hh, f).reshape(24, 128).T)
    return dict(whh=whh, wih=wih, wsum=wsum, wemb=wemb, wobs=wobs, embt=embt,
                bih=bih_t, bhh=bhh_t)


def _prep_core(buoy_obs, buoy_ids, nbuoy):
    f = np.float32
    o = np.asarray(buoy_obs, f)
    ids = np.asarray(buoy_ids)
    # steps 1-3 hit wobs rows 0:64 -> obs in rows 0:64, zeros in 64:128
    # steps 4-5 hit wobs rows 64:128 -> zeros in 0:64, obs in 64:128
    obs0 = np.zeros((128, nbuoy), f)
    obs0[:64] = o[:, 0, :].T
    obs23 = np.zeros((2, 128, nbuoy), f)
    obs45 = np.zeros((2, 128, nbuoy), f)
    for s in range(2):
        obs23[s, :64] = o[:, s + 1, :].T
        obs45[s, 64:] = o[:, s + 1, :].T
    onehot = np.zeros((128, nbuoy), f)
    onehot[ids, np.arange(nbuoy)] = 1.0
    return dict(obs0=obs0, obs23=obs23, obs45=obs45, onehot=onehot)


_NC_CACHE = {}


def _get_nc(nbuoy, blk):
    key = (nbuoy, blk)
    if key not in _NC_CACHE:
        _NC_CACHE[key] = build(nbuoy, blk)
    return _NC_CACHE[key]


def kernel(buoy_obs, buoy_ids, emb, W_ih, W_hh, b_ih, b_hh):
    buoy_obs = np.asarray(buoy_obs)
    buoy_ids = np.asarray(buoy_ids)
    n = buoy_obs.shape[0]
    per = n // N_CORES
    shared = _prep_shared(emb, W_ih, W_hh, b_ih, b_hh)
    in_maps = []
    for c in range(N_CORES):
        sl = slice(c * per, (c + 1) * per)
        m = dict(shared)
        m.update(_prep_core(buoy_obs[sl], buoy_ids[sl], per))
        in_maps.append(m)

    nc = _get_nc(per, 1024)
    res = run_bass_kernel_spmd(nc, in_maps, list(range(N_CORES)))
    outs = []
    for c in range(N_CORES):
        r = res.results[c]["out_t"]                    # [8, 128, per]
        outs.append(r.transpose(2, 0, 1).reshape(per, H))
    full = np.concatenate(outs, axis=0).astype(np.float32)
    return full[None, :, :]



# revision 3
# speedup vs baseline: 1.2351x; 1.2351x over previous
"""Trainium2 Bass kernel for nn_ContextualEncoder2 (5-step GRU over buoys).

Strategy (data-parallel over 16384 buoys across 8 cores, 2048 each):
  * Transposed compute layout: gate-features on SBUF partitions, buoys on
    the free axis; h stays [128, 512] bf16 f-tiles in SBUF between steps.
  * cuDNN-style input-projection precompute: every x-side projection that
    depends only on raw inputs (obs slices through W_ih[:, :64] /
    W_ih[:, 1024:1088] and the 100-row embedding through W_ih[:, 64:1088])
    is computed on host and DMA'd as per-step bf16 "gi" tiles. The device
    performs the full 5-step recurrence: all W_hh @ h / W_ih @ h1|h4
    contractions and every gate nonlinearity.
  * All PE work is K=128 bf16 chains into fp32 PSUM, N=512 moving tiles.
    bf16 stationaries enable FWL (2x faster LDWEIGHTS) so weight loads
    hide fully under the 512-cycle matmuls.
  * Step 5 uses W_hh + W_ih[:, :1024] summed on host for the r/z gates
    (both consume h4), saving one full contraction.
  * Lanes of 512 buoys are processed in pairs per step so weight slabs
    stream once per pair and the PE never waits on elementwise drains;
    step 2 starts with a single-lane pass so the PE fills ~11us in.
"""
import numpy as np
import ml_dtypes

import concourse.bass as bass
import concourse.mybir as mybir
import concourse.tile as tile
from concourse import bacc
from concourse.bass_utils import run_bass_kernel_spmd

F32 = mybir.dt.float32
BF16 = mybir.dt.bfloat16
AF = mybir.ActivationFunctionType
OP = mybir.AluOpType

N_CORES = 8
NUM_BUOYS = 16384
H = 1024
KCH = 8          # 1024 / 128 contraction chunks
FCH = 8          # 1024 / 128 h feature tiles
NT = 512         # moving/free tile width (one PSUM bank of fp32)


def _accum(nc, psum, pairs):
    last = len(pairs) - 1
    for i, (l, r) in enumerate(pairs):
        nc.tensor.matmul(psum, l, r, start=(i == 0), stop=(i == last))


def build(nbuoy=2048):
    """Build the per-core Bass program (same NEFF on every core)."""
    assert nbuoy % NT == 0
    NL = nbuoy // NT

    nc = bacc.Bacc("TRN2", target_bir_lowering=False, debug=False)

    whh = nc.declare_dram_parameter("whh", [24, 128, 1024], BF16, isOutput=False)
    wih = nc.declare_dram_parameter("wih", [24, 128, 1024], BF16, isOutput=False)
    wsum = nc.declare_dram_parameter("wsum", [16, 128, 1024], BF16, isOutput=False)
    gi = nc.declare_dram_parameter("gi", [5, FCH, NL, 128, 3 * NT], BF16,
                                   isOutput=False)
    bih = nc.declare_dram_parameter("bih", [128, 24], F32, isOutput=False)
    bhh = nc.declare_dram_parameter("bhh", [128, 24], F32, isOutput=False)
    out_t = nc.declare_dram_parameter("out_t", [FCH, 128, nbuoy], F32,
                                      isOutput=True)

    whh_ap, wih_ap, wsum_ap, gi_ap, out_ap = (
        whh.ap(), wih.ap(), wsum.ap(), gi.ap(), out_t.ap())

    # lane passes per step: step 2 ramps with single lanes so the PE can
    # start while step-1 elementwise for later lanes is still draining.
    pair_passes = [list(range(i, min(i + 2, NL))) for i in range(0, NL, 2)]
    s2_passes = ([[0], [1]] + [list(range(i, min(i + 2, NL)))
                               for i in range(2, NL, 2)]
                 if NL > 1 else [[0]])

    with tile.TileContext(nc) as tc:
        with (
            tc.tile_pool(name="const", bufs=1) as cpool,
            tc.tile_pool(name="hA", bufs=1) as hApool,      # h1
            tc.tile_pool(name="hB", bufs=1) as hBpool,      # h2 then h4
            tc.tile_pool(name="hC", bufs=1) as hCpool,      # h3
            tc.tile_pool(name="gip", bufs=4) as gpool,
            tc.tile_pool(name="wst", bufs=2) as spool,
            tc.tile_pool(name="work", bufs=2) as wpool,
            tc.tile_pool(name="ps", bufs=1, space="PSUM") as pspool,
        ):
            bih_sb = cpool.tile([128, 24], F32, tag="bih")
            nc.sync.dma_start(bih_sb[:], bih.ap())
            bhh_sb = cpool.tile([128, 24], F32, tag="bhh")
            nc.sync.dma_start(bhh_sb[:], bhh.ap())
            bsum = cpool.tile([128, 24], F32, tag="bsum")
            nc.vector.tensor_add(bsum[:], bih_sb[:], bhh_sb[:])

            def gi_tile(s, f, jj):
                t = gpool.tile([128, 3 * NT], BF16, tag="gi", name="git")
                nc.sync.dma_start(t[:], gi_ap[s][f][jj])
                return t

            def ttile(dtype=F32, tag="t", bufs=None):
                return wpool.tile([128, NT], dtype, tag=tag, name=tag)

            h1 = {}
            h = {}

            # ---- step 1: pure elementwise from hosted gi1 ----------------
            with nc.named_scope("s1"):
                for jj in range(NL):
                    for f in range(FCH):
                        mr, mz, mn = f, 8 + f, 16 + f
                        g = gi_tile(0, f, jj)
                        r = ttile(tag="r")
                        nc.scalar.activation(r[:], g[:, 0:NT], AF.Sigmoid,
                                             bias=bsum[:, mr:mr + 1])
                        z = ttile(tag="z")
                        nc.scalar.activation(z[:], g[:, NT:2 * NT], AF.Sigmoid,
                                             bias=bsum[:, mz:mz + 1])
                        t2 = ttile(tag="t")
                        nc.vector.scalar_tensor_tensor(
                            t2[:], r[:], bhh_sb[:, mn:mn + 1],
                            g[:, 2 * NT:3 * NT], OP.mult, OP.add)
                        n_t = ttile(tag="n")
                        nc.scalar.activation(n_t[:], t2[:], AF.Tanh,
                                             bias=bih_sb[:, mn:mn + 1])
                        e = ttile(tag="e")
                        nc.vector.tensor_mul(e[:], z[:], n_t[:])
                        hn = hApool.tile([128, NT], BF16, tag=f"hA_{f}_{jj}",
                                         name="hn")
                        nc.vector.tensor_sub(hn[:], n_t[:], e[:])
                        h1[(f, jj)] = hn
                        h[(f, jj)] = hn

            # ---- steps 2-5 ----------------------------------------------
            def wslab(src_ap, m, role):
                t = spool.tile([128, 1024], BF16, tag=f"w{role}", name="wsl")
                nc.sync.dma_start(t[:], src_ap[m])
                return t

            def wmm(w, col):
                return [(w[:, k * 128:(k + 1) * 128], col[k][:])
                        for k in range(KCH)]

            for s in (2, 3, 4, 5):
                prev_h = h
                hnew = {}
                newpool, fam = {2: (hBpool, "hB"), 3: (hCpool, "hC"),
                                4: (hBpool, "hB"), 5: (None, None)}[s]
                passes = s2_passes if s == 2 else pair_passes
                with nc.named_scope(f"s{s}"):
                    for lanes in passes:
                        for f in range(FCH):
                            mr, mz, mn = f, 8 + f, 16 + f
                            if s in (2, 3):
                                wr = wslab(whh_ap, mr, "r")
                                wz = wslab(whh_ap, mz, "z")
                                wn = wslab(whh_ap, mn, "n")
                            elif s == 4:
                                wr = wslab(whh_ap, mr, "r")
                                wz = wslab(whh_ap, mz, "z")
                                wn = wslab(whh_ap, mn, "n")
                                vr = wslab(wih_ap, mr, "vr")
                                vz = wslab(wih_ap, mz, "vz")
                                vn = wslab(wih_ap, mn, "vn")
                            else:
                                wr = wslab(wsum_ap, mr, "r")
                                wz = wslab(wsum_ap, mz, "z")
                                wn = wslab(whh_ap, mn, "n")
                                vn = wslab(wih_ap, mn, "vn")
                            for jj in lanes:
                                par = jj % 2
                                g = gi_tile(s - 1, f, jj)
                                hcol = [prev_h[(k, jj)] for k in range(KCH)]
                                if s == 4:
                                    h1col = [h1[(k, jj)] for k in range(KCH)]

                                pr = pspool.tile([128, NT], F32, tag=f"pr{par}",
                                                 name="pr")
                                pairs = wmm(wr, hcol)
                                if s == 4:
                                    pairs += wmm(vr, h1col)
                                _accum(nc, pr[:], pairs)

                                pz = pspool.tile([128, NT], F32, tag=f"pz{par}",
                                                 name="pz")
                                pairs = wmm(wz, hcol)
                                if s == 4:
                                    pairs += wmm(vz, h1col)
                                _accum(nc, pz[:], pairs)

                                pgh = pspool.tile([128, NT], F32,
                                                  tag=f"pg{par}", name="pgh")
                                _accum(nc, pgh[:], wmm(wn, hcol))

                                if s >= 4:
                                    pgi = pspool.tile([128, NT], F32,
                                                      tag=f"pi{par}", name="pgi")
                                    _accum(nc, pgi[:],
                                           wmm(vn, h1col if s == 4 else hcol))

                                tr = ttile(tag="t")
                                nc.vector.tensor_add(tr[:], pr[:], g[:, 0:NT])
                                r = ttile(tag="r")
                                nc.scalar.activation(r[:], tr[:], AF.Sigmoid,
                                                     bias=bsum[:, mr:mr + 1])
                                tz = ttile(tag="t")
                                nc.vector.tensor_add(tz[:], pz[:],
                                                     g[:, NT:2 * NT])
                                z = ttile(tag="z")
                                nc.scalar.activation(z[:], tz[:], AF.Sigmoid,
                                                     bias=bsum[:, mz:mz + 1])
                                # t1 = (gh_n + b_hh_n) * r
                                t1 = ttile(tag="t")
                                nc.vector.scalar_tensor_tensor(
                                    t1[:], pgh[:], bhh_sb[:, mn:mn + 1], r[:],
                                    OP.add, OP.mult)
                                if s >= 4:
                                    t2a = ttile(tag="t")
                                    nc.vector.tensor_add(t2a[:], t1[:], pgi[:])
                                else:
                                    t2a = t1
                                t2 = ttile(tag="t")
                                nc.vector.tensor_add(t2[:], t2a[:],
                                                     g[:, 2 * NT:3 * NT])
                                n_t = ttile(tag="n")
                                nc.scalar.activation(n_t[:], t2[:], AF.Tanh,
                                                     bias=bih_sb[:, mn:mn + 1])
                                d = ttile(tag="d")
                                nc.vector.tensor_sub(
                                    d[:], prev_h[(f, jj)][:], n_t[:])
                                e = ttile(tag="e")
                                nc.vector.tensor_mul(e[:], z[:], d[:])
                                if s < 5:
                                    hn = newpool.tile([128, NT], BF16,
                                                      tag=f"{fam}_{f}_{jj}",
                                                      name="hn")
                                    nc.vector.tensor_add(hn[:], n_t[:], e[:])
                                    hnew[(f, jj)] = hn
                                else:
                                    ho = ttile(tag="ho")
                                    nc.vector.tensor_add(ho[:], n_t[:], e[:])
                                    nc.sync.dma_start(
                                        out_ap[f][:, jj * NT:(jj + 1) * NT],
                                        ho[:])
                    if s < 5:
                        h = hnew

    nc.compile()
    return nc


# ---------------------------------------------------------------------------
# host-side prep / sharding
# ---------------------------------------------------------------------------

def _prep_shared(emb, W_ih, W_hh, b_ih, b_hh):
    f = np.float32
    W_ih = np.asarray(W_ih, f)
    W_hh = np.asarray(W_hh, f)
    emb = np.asarray(emb, f)

    def slabs(W):  # (3072, 1024) -> [24, 128, 1024]: [m, i, k*128+j] = W[128m+j, 128k+i]
        t = W.reshape(24, 128, 8, 128)          # [m, j, k, i]
        return np.ascontiguousarray(t.transpose(0, 3, 2, 1).reshape(24, 128, 1024))

    bf = ml_dtypes.bfloat16
    whh = slabs(W_hh).astype(bf)
    wih = slabs(W_ih[:, :1024]).astype(bf)
    wsum = np.ascontiguousarray(
        slabs(W_hh + W_ih[:, :1024])[:16]).astype(bf)
    emb_proj = emb @ W_ih[:, 64:1088].T          # [100, 3072]
    wobs_a = np.ascontiguousarray(W_ih[:, :64])          # steps 1-3 obs slice
    wobs_b = np.ascontiguousarray(W_ih[:, 1024:1088])    # steps 4-5 obs slice
    bih_t = np.ascontiguousarray(np.asarray(b_ih, f).reshape(24, 128).T)
    bhh_t = np.ascontiguousarray(np.asarray(b_hh, f).reshape(24, 128).T)
    shared = dict(whh=whh, wih=wih, wsum=wsum, bih=bih_t, bhh=bhh_t)
    return shared, emb_proj.astype(f), wobs_a, wobs_b


def _prep_core(buoy_obs, buoy_ids, emb_proj, wobs_a, wobs_b, nbuoy):
    """Host-side input projections: gi[s] = x_obs/emb part of W_ih @ x_s."""
    f = np.float32
    o = np.asarray(buoy_obs, f)
    ids = np.asarray(buoy_ids)
    ep = emb_proj[ids]                          # [nb, 3072]
    gi = np.empty((5, 3072, nbuoy), f)
    gi[0] = (o[:, 0, :] @ wobs_a.T + ep).T
    gi[1] = (o[:, 1, :] @ wobs_a.T + ep).T
    gi[2] = (o[:, 2, :] @ wobs_a.T + ep).T
    gi[3] = (o[:, 1, :] @ wobs_b.T).T
    gi[4] = (o[:, 2, :] @ wobs_b.T).T
    NL = nbuoy // NT
    # [s, gate, f, p, jj, c] -> [s, f, jj, p, gate*NT + c]
    g = gi.reshape(5, 3, 8, 128, NL, NT).transpose(0, 2, 4, 3, 1, 5)
    g = np.ascontiguousarray(g).reshape(5, 8, NL, 128, 3 * NT)
    return dict(gi=g.astype(ml_dtypes.bfloat16))


_NC_CACHE = {}


def _get_nc(nbuoy):
    if nbuoy not in _NC_CACHE:
        _NC_CACHE[nbuoy] = build(nbuoy)
    return _NC_CACHE[nbuoy]


def kernel(buoy_obs, buoy_ids, emb, W_ih, W_hh, b_ih, b_hh):
    buoy_obs = np.asarray(buoy_obs)
    buoy_ids = np.asarray(buoy_ids)
    n = buoy_obs.shape[0]
    per = n // N_CORES
    shared, emb_proj, wobs_a, wobs_b = _prep_shared(emb, W_ih, W_hh, b_ih, b_hh)
    in_maps = []
    for c in range(N_CORES):
        sl = slice(c * per, (c + 1) * per)
        m = dict(shared)
        m.update(_prep_core(buoy_obs[sl], buoy_ids[sl],
                            emb_proj, wobs_a, wobs_b, per))
        in_maps.append(m)

    nc = _get_nc(per)
    res = run_bass_kernel_spmd(nc, in_maps, list(range(N_CORES)))
    outs = []
    for c in range(N_CORES):
        r = res.results[c]["out_t"]                    # [8, 128, per]
        outs.append(np.asarray(r, np.float32).transpose(2, 0, 1).reshape(per, H))
    full = np.concatenate(outs, axis=0).astype(np.float32)
    return full[None, :, :]


# revision 6
# speedup vs baseline: 1.2397x; 1.0037x over previous
"""Trainium2 Bass kernel for nn_ContextualEncoder2 (5-step GRU over buoys).

Strategy (data-parallel over 16384 buoys across 8 cores, 2048 each):
  * Transposed compute layout: gate-features on SBUF partitions, buoys on
    the free axis; h stays [128, 512] bf16 f-tiles in SBUF between steps.
  * cuDNN-style input-projection precompute: every x-side projection that
    depends only on raw inputs (obs slices through W_ih[:, :64] /
    W_ih[:, 1024:1088] and the 100-row embedding through W_ih[:, 64:1088])
    is computed on host and DMA'd as per-step bf16 "gi" tiles. The device
    performs the full 5-step recurrence: all W_hh @ h / W_ih @ h1|h4
    contractions and every gate nonlinearity.
  * All PE work is K=128 bf16 chains into fp32 PSUM, N=512 moving tiles.
    bf16 stationaries enable FWL (2x faster LDWEIGHTS) so weight loads
    hide fully under the 512-cycle matmuls.
  * Step 5 uses W_hh + W_ih[:, :1024] summed on host for the r/z gates
    (both consume h4), saving one full contraction.
  * Lanes of 512 buoys are processed in pairs per step so weight slabs
    stream once per pair and the PE never waits on elementwise drains;
    step 2 starts with a single-lane pass so the PE fills ~11us in.
"""
import numpy as np

import concourse.bass as bass
import concourse.mybir as mybir
import concourse.tile as tile
from concourse import bacc
from concourse.bass_utils import run_bass_kernel_spmd

F32 = mybir.dt.float32
F16 = mybir.dt.float16
AF = mybir.ActivationFunctionType
OP = mybir.AluOpType

N_CORES = 8
NUM_BUOYS = 16384
H = 1024
KCH = 8          # 1024 / 128 contraction chunks
FCH = 8          # 1024 / 128 h feature tiles
NT = 512         # moving/free tile width (one PSUM bank of fp32)


def _accum(nc, psum, pairs):
    last = len(pairs) - 1
    for i, (l, r) in enumerate(pairs):
        nc.tensor.matmul(psum, l, r, start=(i == 0), stop=(i == last))


def build(nbuoy=2048):
    """Build the per-core Bass program (same NEFF on every core)."""
    assert nbuoy % NT == 0
    NL = nbuoy // NT

    nc = bacc.Bacc("TRN2", target_bir_lowering=False, debug=False)

    whh = nc.declare_dram_parameter("whh", [24, 128, 1024], F16, isOutput=False)
    wih = nc.declare_dram_parameter("wih", [24, 128, 1024], F16, isOutput=False)
    wsum = nc.declare_dram_parameter("wsum", [16, 128, 1024], F16, isOutput=False)
    gi = nc.declare_dram_parameter("gi", [5, FCH, NL, 128, 3 * NT], F16,
                                   isOutput=False)
    bih = nc.declare_dram_parameter("bih", [128, 24], F32, isOutput=False)
    bhh = nc.declare_dram_parameter("bhh", [128, 24], F32, isOutput=False)
    out_t = nc.declare_dram_parameter("out_t", [FCH, 128, nbuoy], F16,
                                      isOutput=True)

    whh_ap, wih_ap, wsum_ap, gi_ap, out_ap = (
        whh.ap(), wih.ap(), wsum.ap(), gi.ap(), out_t.ap())

    # lane passes per step: step 2 ramps with single lanes so the PE can
    # start while step-1 elementwise for later lanes is still draining.
    pair_passes = [list(range(i, min(i + 2, NL))) for i in range(0, NL, 2)]
    s2_passes = ([[0], [1]] + [list(range(i, min(i + 2, NL)))
                               for i in range(2, NL, 2)]
                 if NL > 1 else [[0]])

    with tile.TileContext(nc) as tc:
        with (
            tc.tile_pool(name="const", bufs=1) as cpool,
            tc.tile_pool(name="hA", bufs=1) as hApool,      # h1
            tc.tile_pool(name="hB", bufs=1) as hBpool,      # h2 then h4
            tc.tile_pool(name="hC", bufs=1) as hCpool,      # h3
            tc.tile_pool(name="gip", bufs=4) as gpool,
            tc.tile_pool(name="wst", bufs=2) as spool,
            tc.tile_pool(name="work", bufs=2) as wpool,
            tc.tile_pool(name="ps", bufs=1, space="PSUM") as pspool,
        ):
            bih_sb = cpool.tile([128, 24], F32, tag="bih")
            nc.sync.dma_start(bih_sb[:], bih.ap())
            bhh_sb = cpool.tile([128, 24], F32, tag="bhh")
            nc.sync.dma_start(bhh_sb[:], bhh.ap())
            bsum = cpool.tile([128, 24], F32, tag="bsum")
            nc.vector.tensor_add(bsum[:], bih_sb[:], bhh_sb[:])

            def gi_tile(s, f, jj):
                t = gpool.tile([128, 3 * NT], F16, tag="gi", name="git")
                nc.sync.dma_start(t[:], gi_ap[s][f][jj])
                return t

            def ttile(dtype=F16, tag="t", bufs=None):
                return wpool.tile([128, NT], dtype, tag=tag, name=tag)

            h1 = {}
            h = {}

            # ---- step 1: pure elementwise from hosted gi1 ----------------
            with nc.named_scope("s1"):
                for jj in range(NL):
                    for f in range(FCH):
                        mr, mz, mn = f, 8 + f, 16 + f
                        g = gi_tile(0, f, jj)
                        r = ttile(tag="r")
                        nc.scalar.activation(r[:], g[:, 0:NT], AF.Sigmoid,
                                             bias=bsum[:, mr:mr + 1])
                        z = ttile(tag="z")
                        nc.scalar.activation(z[:], g[:, NT:2 * NT], AF.Sigmoid,
                                             bias=bsum[:, mz:mz + 1])
                        t2 = ttile(tag="t")
                        nc.vector.scalar_tensor_tensor(
                            t2[:], r[:], bhh_sb[:, mn:mn + 1],
                            g[:, 2 * NT:3 * NT], OP.mult, OP.add)
                        n_t = ttile(tag="n")
                        nc.scalar.activation(n_t[:], t2[:], AF.Tanh,
                                             bias=bih_sb[:, mn:mn + 1])
                        e = ttile(tag="e")
                        nc.vector.tensor_mul(e[:], z[:], n_t[:])
                        hn = hApool.tile([128, NT], F16, tag=f"hA_{f}_{jj}",
                                         name="hn")
                        nc.vector.tensor_sub(hn[:], n_t[:], e[:])
                        h1[(f, jj)] = hn
                        h[(f, jj)] = hn

            # ---- steps 2-5 ----------------------------------------------
            def wslab(src_ap, m, role):
                t = spool.tile([128, 1024], F16, tag=f"w{role}", name="wsl")
                nc.sync.dma_start(t[:], src_ap[m])
                return t

            def wmm(w, col):
                return [(w[:, k * 128:(k + 1) * 128], col[k][:])
                        for k in range(KCH)]

            for s in (2, 3, 4, 5):
                prev_h = h
                hnew = {}
                newpool, fam = {2: (hBpool, "hB"), 3: (hCpool, "hC"),
                                4: (hBpool, "hB"), 5: (None, None)}[s]
                passes = s2_passes if s == 2 else pair_passes
                with nc.named_scope(f"s{s}"):
                    for lanes in passes:
                        for f in range(FCH):
                            mr, mz, mn = f, 8 + f, 16 + f
                            if s in (2, 3):
                                wr = wslab(whh_ap, mr, "r")
                                wz = wslab(whh_ap, mz, "z")
                                wn = wslab(whh_ap, mn, "n")
                            elif s == 4:
                                wr = wslab(whh_ap, mr, "r")
                                wz = wslab(whh_ap, mz, "z")
                                wn = wslab(whh_ap, mn, "n")
                                vr = wslab(wih_ap, mr, "vr")
                                vz = wslab(wih_ap, mz, "vz")
                                vn = wslab(wih_ap, mn, "vn")
                            else:
                                wr = wslab(wsum_ap, mr, "r")
                                wz = wslab(wsum_ap, mz, "z")
                                wn = wslab(whh_ap, mn, "n")
                                vn = wslab(wih_ap, mn, "vn")
                            for jj in lanes:
                                par = jj % 2
                                g = gi_tile(s - 1, f, jj)
                                hcol = [prev_h[(k, jj)] for k in range(KCH)]
                                if s == 4:
                                    h1col = [h1[(k, jj)] for k in range(KCH)]

                                pr = pspool.tile([128, NT], F32, tag=f"pr{par}",
                                                 name="pr")
                                pairs = wmm(wr, hcol)
                                if s == 4:
                                    pairs += wmm(vr, h1col)
                                _accum(nc, pr[:], pairs)

                                pz = pspool.tile([128, NT], F32, tag=f"pz{par}",
                                                 name="pz")
                                pairs = wmm(wz, hcol)
                                if s == 4:
                                    pairs += wmm(vz, h1col)
                                _accum(nc, pz[:], pairs)

                                pgh = pspool.tile([128, NT], F32,
                                                  tag=f"pg{par}", name="pgh")
                                _accum(nc, pgh[:], wmm(wn, hcol))

                                if s >= 4:
                                    pgi = pspool.tile([128, NT], F32,
                                                      tag=f"pi{par}", name="pgi")
                                    _accum(nc, pgi[:],
                                           wmm(vn, h1col if s == 4 else hcol))

                                tr = ttile(tag="t")
                                nc.vector.tensor_add(tr[:], pr[:], g[:, 0:NT])
                                r = ttile(tag="r")
                                nc.scalar.activation(r[:], tr[:], AF.Sigmoid,
                                                     bias=bsum[:, mr:mr + 1])
                                tz = ttile(tag="t")
                                nc.vector.tensor_add(tz[:], pz[:],
                                                     g[:, NT:2 * NT])
                                z = ttile(tag="z")
                                nc.scalar.activation(z[:], tz[:], AF.Sigmoid,
                                                     bias=bsum[:, mz:mz + 1])
                                # t1 = (gh_n + b_hh_n) * r
                                t1 = ttile(tag="t")
                                nc.vector.scalar_tensor_tensor(
                                    t1[:], pgh[:], bhh_sb[:, mn:mn + 1], r[:],
                                    OP.add, OP.mult)
                                if s >= 4:
                                    t2a = ttile(tag="t")
                                    nc.vector.tensor_add(t2a[:], t1[:], pgi[:])
                                else:
                                    t2a = t1
                                t2 = ttile(tag="t")
                                nc.vector.tensor_add(t2[:], t2a[:],
                                                     g[:, 2 * NT:3 * NT])
                                n_t = ttile(tag="n")
                                nc.scalar.activation(n_t[:], t2[:], AF.Tanh,
                                                     bias=bih_sb[:, mn:mn + 1])
                                d = ttile(tag="d")
                                nc.vector.tensor_sub(
                                    d[:], prev_h[(f, jj)][:], n_t[:])
                                e = ttile(tag="e")
                                nc.vector.tensor_mul(e[:], z[:], d[:])
                                if s < 5:
                                    hn = newpool.tile([128, NT], F16,
                                                      tag=f"{fam}_{f}_{jj}",
                                                      name="hn")
                                    nc.vector.tensor_add(hn[:], n_t[:], e[:])
                                    hnew[(f, jj)] = hn
                                else:
                                    ho = ttile(tag="ho")
                                    nc.vector.tensor_add(ho[:], n_t[:], e[:])
                                    nc.sync.dma_start(
                                        out_ap[f][:, jj * NT:(jj + 1) * NT],
                                        ho[:])
                    if s < 5:
                        h = hnew

    nc.compile()
    return nc


# ---------------------------------------------------------------------------
# host-side prep / sharding
# ---------------------------------------------------------------------------

def _prep_shared(emb, W_ih, W_hh, b_ih, b_hh):
    f = np.float32
    W_ih = np.asarray(W_ih, f)
    W_hh = np.asarray(W_hh, f)
    emb = np.asarray(emb, f)

    def slabs(W):  # (3072, 1024) -> [24, 128, 1024]: [m, i, k*128+j] = W[128m+j, 128k+i]
        t = W.reshape(24, 128, 8, 128)          # [m, j, k, i]
        return np.ascontiguousarray(t.transpose(0, 3, 2, 1).reshape(24, 128, 1024))

    whh = slabs(W_hh).astype(np.float16)
    wih = slabs(W_ih[:, :1024]).astype(np.float16)
    wsum = np.ascontiguousarray(
        slabs(W_hh + W_ih[:, :1024])[:16]).astype(np.float16)
    emb_proj = emb @ W_ih[:, 64:1088].T          # [100, 3072]
    wobs_a = np.ascontiguousarray(W_ih[:, :64])          # steps 1-3 obs slice
    wobs_b = np.ascontiguousarray(W_ih[:, 1024:1088])    # steps 4-5 obs slice
    bih_t = np.ascontiguousarray(np.asarray(b_ih, f).reshape(24, 128).T)
    bhh_t = np.ascontiguousarray(np.asarray(b_hh, f).reshape(24, 128).T)
    shared = dict(whh=whh, wih=wih, wsum=wsum, bih=bih_t, bhh=bhh_t)
    return shared, emb_proj.astype(f), wobs_a, wobs_b


def _prep_core(buoy_obs, buoy_ids, emb_proj, wobs_a, wobs_b, nbuoy):
    """Host-side input projections: gi[s] = x_obs/emb part of W_ih @ x_s."""
    f = np.float32
    o = np.asarray(buoy_obs, f)
    ids = np.asarray(buoy_ids)
    ep = emb_proj[ids]                          # [nb, 3072]
    gi = np.empty((5, 3072, nbuoy), f)
    gi[0] = (o[:, 0, :] @ wobs_a.T + ep).T
    gi[1] = (o[:, 1, :] @ wobs_a.T + ep).T
    gi[2] = (o[:, 2, :] @ wobs_a.T + ep).T
    gi[3] = (o[:, 1, :] @ wobs_b.T).T
    gi[4] = (o[:, 2, :] @ wobs_b.T).T
    NL = nbuoy // NT
    # [s, gate, f, p, jj, c] -> [s, f, jj, p, gate*NT + c]
    g = gi.reshape(5, 3, 8, 128, NL, NT).transpose(0, 2, 4, 3, 1, 5)
    g = np.ascontiguousarray(g).reshape(5, 8, NL, 128, 3 * NT)
    return dict(gi=g.astype(np.float16))


_NC_CACHE = {}


def _get_nc(nbuoy):
    if nbuoy not in _NC_CACHE:
        _NC_CACHE[nbuoy] = build(nbuoy)
    return _NC_CACHE[nbuoy]


def kernel(buoy_obs, buoy_ids, emb, W_ih, W_hh, b_ih, b_hh):
    buoy_obs = np.asarray(buoy_obs)
    buoy_ids = np.asarray(buoy_ids)
    n = buoy_obs.shape[0]
    per = n // N_CORES
    shared, emb_proj, wobs_a, wobs_b = _prep_shared(emb, W_ih, W_hh, b_ih, b_hh)
    in_maps = []
    for c in range(N_CORES):
        sl = slice(c * per, (c + 1) * per)
        m = dict(shared)
        m.update(_prep_core(buoy_obs[sl], buoy_ids[sl],
                            emb_proj, wobs_a, wobs_b, per))
        in_maps.append(m)

    nc = _get_nc(per)
    res = run_bass_kernel_spmd(nc, in_maps, list(range(N_CORES)))
    outs = []
    for c in range(N_CORES):
        r = res.results[c]["out_t"]                    # [8, 128, per]
        outs.append(np.asarray(r, np.float32).transpose(2, 0, 1).reshape(per, H))
    full = np.concatenate(outs, axis=0).astype(np.float32)
    return full[None, :, :]
